# revision 9
# baseline (speedup 1.0000x reference)
"""Trainium2 Bass kernel for nn_CNN_88098369175781.

Model: x[1,1,18,T=262144] -> wavA=x[...,0,:], eeg=x[...,1:17,:], wavB=x[...,17,:]
  wav streams: proj(1->16, pointwise) -> diagonal sinc filter bank (15 taps,
  pad 7) -> conv(16->10, 9 taps) + bias -> relu -> global max-pool.
  eeg stream:  conv(16->10, 9 taps) + bias -> relu -> global max-pool.
  concat -> sigmoid FC(30->30) -> sigmoid FC(30->2).

Device decomposition:
  * Wav streams fuse to ONE 1->10ch 23-tap conv (precomposed host-side).
  * Bias/relu commute past the global max; device computes convs + maxima.
  * eeg conv via B=12 polyphase: out[o, s+12m+dt], M=120=(o,dt), three
    accumulating passes: A (s'=r in [0,8), K=128), C (s'=12+r from the
    m+1 column, K=128), B (s'=8+r' in [8,12), K=64).
  * wav via 12-phase polyphase (K=36, M=120); the A and B streams run as
    row-tiled concurrent matmuls (tile_position (0,0) / (64,0)).
  * PSUM evacuation: ACT casts some banks to fp16 SBUF; GpSimd folds the
    stream tails into those casts in-place (tensor_tensor max); DVE
    tensor_tensor_reduce consumes a fresh PSUM bank and a cast tile per
    pass, writing per-row maxima into the output tile.
  * 8 cores split the time axis (overlapping chunks; overlap free for max).
  * Host combines per-core maxima and runs the tiny FC head.
"""

import os
import numpy as np

T = 262144
NOUT = T - 8            # 262136 valid conv output positions
NCORES = 8
KLEN = 15
SIGMA = 0.005

B12 = 12
NCOL = 2731             # phase columns per core (12*2731 = 32772 outputs)
TC = 12 * NCOL          # outputs per core

_NC_CACHE = {}
LAST_RESULT = None      # BassKernelResults of the most recent device run


# --------------------------------------------------------------------------
# host-side weight precompute
# --------------------------------------------------------------------------

def _sinc_rows(mu):
    """Diagonal rows of the reference's sinc_kernel: [16, 15] float64."""
    k = np.linspace(-1.0, 1.0, KLEN)
    kk = (k[None, :] - np.asarray(mu, np.float64)[:, None]) / SIGMA
    nos = np.sum(np.abs(kk) < 1e-5, axis=1)
    kk = np.where((nos >= 0.5)[:, None], kk - 5e-5, kk)
    return np.sin(np.pi * kk) / (np.pi * kk)


def _composite_wav_weights(mu, proj_w, conv_w_i):
    """Fused 1->10ch 23-tap kernel E[o, s] (float64)."""
    krn = _sinc_rows(mu)                                  # [16,15]
    a = np.asarray(proj_w, np.float64)[:, 0, 0]           # [16]
    W = np.asarray(conv_w_i, np.float64)                  # [10,16,9]
    E = np.zeros((10, 23))
    for j in range(9):
        E[:, j:j + 15] += np.einsum('oc,cm->om', W[:, :, j] * a[None, :], krn)
    return E


def _eeg_lhsT12(W1):
    """B=12 eeg weights: (W_A [128,120], W_C [128,120], W_B [64,120]).

    cols (o*12+dt); A rows (c*8+r): W1[o,c,r-dt]; C rows (c*8+r):
    W1[o,c,12+r-dt]; B rows (c*4+r'): W1[o,c,8+r'-dt]."""
    W1 = np.asarray(W1, np.float64)         # [10,16,9]
    WA = np.zeros((128, 120))
    WC = np.zeros((128, 120))
    WB = np.zeros((64, 120))
    for o in range(10):
        for dt in range(12):
            col = o * 12 + dt
            for c in range(16):
                for r in range(8):
                    j = r - dt
                    if 0 <= j < 9:
                        WA[c * 8 + r, col] = W1[o, c, j]
                    j = 12 + r - dt
                    if 0 <= j < 9:
                        WC[c * 8 + r, col] = W1[o, c, j]
                for rp in range(4):
                    j = 8 + rp - dt
                    if 0 <= j < 9:
                        WB[c * 4 + rp, col] = W1[o, c, j]
    return (WA.astype(np.float32), WC.astype(np.float32),
            WB.astype(np.float32))


def _wav_lhsT(E):
    """[36, 120]: row v*3+q, col o*12+dt, val E[o, 12q+v-dt]."""
    out = np.zeros((36, 120))
    v, q, o, dt = np.meshgrid(np.arange(12), np.arange(3), np.arange(10),
                              np.arange(12), indexing='ij')
    s = 12 * q + v - dt
    valid = (s >= 0) & (s < 23)
    out[(v * 3 + q)[valid], (o * 12 + dt)[valid]] = E[o[valid], np.clip(s[valid], 0, 22)]
    return out.astype(np.float32)


# --------------------------------------------------------------------------
# host-side per-core input slicing
# --------------------------------------------------------------------------

def _core_start(k):
    return min(k * 32767, NOUT - TC)


def _eeg_phases12(eegp, k):
    """eegp: [16, T+pad] fp16. Returns (plo [128, 2732], phi [64, 2731])."""
    s = _core_start(k)
    v = eegp[:, s:s + 12 * (NCOL + 1)]                    # [16, 32784]
    p = v.reshape(16, NCOL + 1, 12)                       # [16, 2732, 12]
    plo = p[:, :, 0:8].transpose(0, 2, 1).reshape(128, NCOL + 1)
    phi = p[:, 0:NCOL, 8:12].transpose(0, 2, 1).reshape(64, NCOL)
    return np.ascontiguousarray(plo), np.ascontiguousarray(phi)


def _wav_phases(w_pad, k):
    """[36, 2731]: row v*3+q, col n = w_pad[s + 12(n+q) + v]."""
    s = _core_start(k)
    sl = w_pad[s:s + 12 * (NCOL + 2)]
    y = sl.reshape(NCOL + 2, 12).T                        # y[v,m] = sl[12m+v]
    out = np.empty((36, NCOL), dtype=w_pad.dtype)
    for q in range(3):
        out[q::3, :] = y[:, q:q + NCOL]
    return np.ascontiguousarray(out)


# --------------------------------------------------------------------------
# bass kernel
# --------------------------------------------------------------------------

def _build_nc():
    import concourse.bacc as bacc
    import concourse.tile as tile
    import concourse.mybir as mybir

    f32 = mybir.dt.float32
    f16 = mybir.dt.float16
    Max = mybir.AluOpType.max
    X = mybir.AxisListType.X
    Copy = mybir.ActivationFunctionType.Copy

    no_ttr = bool(os.environ.get("KV_NO_TTR"))
    no_tilepos = bool(os.environ.get("KV_NO_TILEPOS"))
    nc = bacc.Bacc("TRN2", target_bir_lowering=False, debug=False,
                   num_devices=NCORES)

    # DRAM I/O
    wts = nc.dram_tensor("wts", [128, 480], f16, kind="ExternalInput")
    plo = nc.dram_tensor("plo", [128, NCOL + 1], f16, kind="ExternalInput")
    phi = nc.dram_tensor("phi", [64, NCOL], f16, kind="ExternalInput")
    wavA = nc.dram_tensor("wavA", [36, NCOL], f16, kind="ExternalInput")
    wavB = nc.dram_tensor("wavB", [36, NCOL], f16, kind="ExternalInput")
    out = nc.dram_tensor("out", [128, 8], f16, kind="ExternalOutput")

    NINIT = -60000.0
    N_WARM = 4

    with tile.TileContext(nc) as tc:
        with (
            tc.tile_pool(name="sb", bufs=1) as sb,
            tc.tile_pool(name="ps", bufs=4, space="PSUM") as psp,
        ):
            # ---- SBUF tiles
            scr = sb.tile([128, 512], f16, tag="scr")
            wtsT = sb.tile([128, 480], f16, tag="wtsT")
            ploT0 = sb.tile([128, 1025], f16, tag="ploT0")
            ploT1 = sb.tile([128, 1025], f16, tag="ploT1")
            ploT2 = sb.tile([128, 684], f16, tag="ploT2")
            phiT0 = sb.tile([64, 1024], f16, tag="phiT0")
            phiT1 = sb.tile([64, 1024], f16, tag="phiT1")
            phiT2 = sb.tile([64, 683], f16, tag="phiT2")
            wavTA = sb.tile([36, NCOL], f16, tag="wavTA")
            wavTB = sb.tile([128, NCOL], f16, tag="wavTB")  # rows 64..99 used
            cE0 = sb.tile([120, 1024], f16, tag="cE0")
            cA0 = sb.tile([120, 1024], f16, tag="cA0")
            cB0 = sb.tile([120, 1024], f16, tag="cB0")
            fscr = sb.tile([120, 1024], f32, tag="fscr")
            out32 = sb.tile([128, 8], f16, tag="out32")

            # ---- input DMAs.  SP ring: wts + eeg P_lo pieces (+ phi tail).
            nc.sync.dma_start(wtsT[:], wts[:])
            nc.sync.dma_start(ploT0[:], plo[:, 0:1025])
            nc.sync.dma_start(ploT1[:], plo[:, 1024:2049])
            nc.sync.dma_start(ploT2[:], plo[:, 2048:2732])
            nc.sync.dma_start(phiT2[:], phi[:, 2048:2731])
            # ACT ring: phi pieces (before the auto-inserted act table load)
            nc.scalar.dma_start(phiT0[:], phi[:, 0:1024])
            nc.scalar.dma_start(phiT1[:], phi[:, 1024:2048])
            # SWDGE: wav streams
            nc.gpsimd.dma_start(wavTA[:], wavA[:])
            nc.gpsimd.dma_start(wavTB[64:100, :], wavB[:])

            # init output partials to -inf-ish; memset warmup scratch
            nc.gpsimd.memset(out32[:], NINIT)
            nc.gpsimd.memset(scr[:], 0.0)

            # ---- PE warmup on scratch (keeps HAM busy while DMAs land)
            psD = psp.tile([120, 1024], f32, tag="u", name="psD")
            for _ in range(N_WARM):
                nc.tensor.matmul(psD[0:80, 0:512], scr[:, 0:80], scr[:],
                                 start=True, stop=True)

            wA = wtsT[:, 0:120]          # eeg pass A (K=128)
            wC = wtsT[:, 120:240]        # eeg pass C (K=128)
            wB = wtsT[0:64, 240:360]     # eeg pass B (K=64)
            wWA = wtsT[0:36, 360:480]    # wav A lhsT
            wWB = wtsT[64:100, 360:480]  # wav B lhsT

            def eeg_unit(ps, pl, lo, n, ph, phlo):
                # pl: plo tile, lo: col offset inside it; ph: phi tile
                for j in range(0, n, 512):
                    w = min(512, n - j)
                    nc.tensor.matmul(ps[:, j:j + w], wA,
                                     pl[:, lo + j:lo + j + w],
                                     start=True, stop=False)
                    nc.tensor.matmul(ps[:, j:j + w], wC,
                                     pl[:, lo + j + 1:lo + j + 1 + w],
                                     start=False, stop=False)
                    nc.tensor.matmul(ps[:, j:j + w], wB,
                                     ph[:, phlo + j:phlo + j + w],
                                     start=False, stop=True)

            def wav_pair(psA, psB, c0, n):
                tpA = None if no_tilepos else (0, 0)
                tpB = None if no_tilepos else (64, 0)
                for j in range(0, n, 512):
                    w = min(512, n - j)
                    nc.tensor.matmul(psA[:, j:j + w], wWA,
                                     wavTA[:, c0 + j:c0 + j + w],
                                     start=True, stop=True,
                                     tile_position=tpA)
                    nc.tensor.matmul(psB[:, j:j + w], wWB,
                                     wavTB[64:100, c0 + j:c0 + j + w],
                                     start=True, stop=True,
                                     tile_position=tpB)

            def ttr(ps_ap, cast_ap, col):
                if no_ttr:
                    nc.vector.tensor_tensor(fscr[:], ps_ap, cast_ap, Max)
                    nc.vector.tensor_reduce(out32[0:120, col:col + 1],
                                            fscr[:], X, Max)
                else:
                    # scan with max/max: final state = max over both tiles
                    nc.vector.tensor_tensor_scan(fscr[:], ps_ap, cast_ap,
                                                 NINIT, Max, Max)
                    nc.vector.tensor_copy(out32[0:120, col:col + 1],
                                          fscr[:, 1023:1024])

            # ---- matmul + evacuation schedule
            # units: E0 E1 E2(683) / A0 A1 A2(683) / B0 B1 B2(683)
            psE0 = psp.tile([120, 1024], f32, tag="u", name="psE0")
            eeg_unit(psE0, ploT0, 0, 1024, phiT0, 0)
            nc.scalar.activation(cE0[:], psE0[:], Copy)

            psE1 = psp.tile([120, 1024], f32, tag="u", name="psE1")
            eeg_unit(psE1, ploT1, 0, 1024, phiT1, 0)

            psA0 = psp.tile([120, 1024], f32, tag="u", name="psA0")
            psB0 = psp.tile([120, 1024], f32, tag="u", name="psB0")
            wav_pair(psA0, psB0, 0, 1024)
            nc.scalar.activation(cA0[:], psA0[:], Copy)

            # TTR: consume psE1 + cE0 -> eeg partial col 0
            ttr(psE1[:], cE0[:], 0)

            psE2 = psp.tile([120, 1024], f32, tag="u", name="psE2")
            eeg_unit(psE2, ploT2, 0, 683, phiT2, 0)
            nc.vector.tensor_reduce(out32[0:120, 1:2], psE2[:, 0:683], X, Max)

            psA1 = psp.tile([120, 1024], f32, tag="u", name="psA1")
            psB1 = psp.tile([120, 1024], f32, tag="u", name="psB1")
            wav_pair(psA1, psB1, 1024, 1024)

            psA2 = psp.tile([120, 1024], f32, tag="u", name="psA2")
            psB2 = psp.tile([120, 1024], f32, tag="u", name="psB2")
            wav_pair(psA2, psB2, 2048, 683)

            # wavB unit 0 cast
            nc.scalar.activation(cB0[:], psB0[:], Copy)

            # TTR: consume psA1 + cA0 -> wavA partial col 2
            ttr(psA1[:], cA0[:], 2)
            ttr(psB1[:], cB0[:], 3)
            nc.vector.tensor_reduce(out32[0:120, 4:5], psA2[:, 0:683], X, Max)
            nc.vector.tensor_reduce(out32[0:120, 5:6], psB2[:, 0:683], X, Max)

            nc.sync.dma_start(out[:], out32[:])

    nc.compile()
    return nc


def _get_nc():
    if "nc" not in _NC_CACHE:
        _NC_CACHE["nc"] = _build_nc()
    return _NC_CACHE["nc"]


# --------------------------------------------------------------------------
# entry point
# --------------------------------------------------------------------------

def _prepare_in_maps(x, mu, projA_w, projB_w, conv_w):
    x = np.asarray(x, np.float32)
    eegp = np.concatenate([x[0, 0, 1:17, :], np.zeros((16, 64), np.float32)],
                          axis=1).astype(np.float16)
    zt = np.zeros(64, np.float32)
    w_padA = np.concatenate([np.zeros(7, np.float32), x[0, 0, 0, :], zt]
                            ).astype(np.float16)
    w_padB = np.concatenate([np.zeros(7, np.float32), x[0, 0, 17, :], zt]
                            ).astype(np.float16)

    conv_w = np.asarray(conv_w)
    E_A = _composite_wav_weights(mu, projA_w, conv_w[0])
    E_B = _composite_wav_weights(mu, projB_w, conv_w[2])
    WA, WC, WB = _eeg_lhsT12(conv_w[1])
    wts_np = np.zeros((128, 480), np.float16)
    wts_np[:, 0:120] = WA
    wts_np[:, 120:240] = WC
    wts_np[0:64, 240:360] = WB
    wts_np[0:36, 360:480] = _wav_lhsT(E_A)
    wts_np[64:100, 360:480] = _wav_lhsT(E_B)

    in_maps = []
    for k in range(NCORES):
        plo_k, phi_k = _eeg_phases12(eegp, k)
        in_maps.append({
            "wts": wts_np,
            "plo": plo_k,
            "phi": phi_k,
            "wavA": _wav_phases(w_padA, k),
            "wavB": _wav_phases(w_padB, k),
        })
    return in_maps


def _head(percore, conv_b, fc1_w, fc1_b, fc2_w, fc2_b):
    m = percore.max(axis=0).astype(np.float64)            # [360]
    eeg_o = m[0:120].reshape(10, 12).max(axis=1)
    wavA_o = m[120:240].reshape(10, 12).max(axis=1)
    wavB_o = m[240:360].reshape(10, 12).max(axis=1)
    conv_b = np.asarray(conv_b, np.float64)
    f = np.concatenate([np.maximum(wavA_o + conv_b[0], 0.0),
                        np.maximum(eeg_o + conv_b[1], 0.0),
                        np.maximum(wavB_o + conv_b[2], 0.0)])
    h = 1.0 / (1.0 + np.exp(-(f @ np.asarray(fc1_w, np.float64).T
                              + np.asarray(fc1_b, np.float64))))
    o = 1.0 / (1.0 + np.exp(-(h @ np.asarray(fc2_w, np.float64).T
                              + np.asarray(fc2_b, np.float64))))
    return o[None, :].astype(np.float32)


def _percore_from_out(arr):
    """Device 'out' [128,8] fp16 -> flat [360] (eeg 120, wavA 120, wavB 120).

    cols 0,1: eeg partials; 2,4: wavA; 3,5: wavB."""
    arr = np.asarray(arr, np.float32)
    return np.concatenate([arr[0:120, [0, 1]].max(axis=1),
                           arr[0:120, [2, 4]].max(axis=1),
                           arr[0:120, [3, 5]].max(axis=1)])


def kernel(x, mu, projA_w, projB_w, conv_w, conv_b, fc1_w, fc1_b, fc2_w, fc2_b):
    global LAST_RESULT
    in_maps = _prepare_in_maps(x, mu, projA_w, projB_w, conv_w)
    nc = _get_nc()

    if os.environ.get("KERNEL_USE_SIM"):
        from concourse.bass_interp import CoreSim
        percore = np.zeros((NCORES, 360), np.float32)
        for k in range(NCORES):
            sim = CoreSim(nc)
            for name, arr in in_maps[k].items():
                sim.tensor(name)[:] = arr
            sim.simulate()
            percore[k] = _percore_from_out(sim.tensor("out"))
    else:
        from concourse.bass_utils import run_bass_kernel_spmd
        trace = bool(os.environ.get("KERNEL_TRACE"))
        res = run_bass_kernel_spmd(nc, in_maps, list(range(NCORES)),
                                   trace=trace)
        LAST_RESULT = res
        percore = np.stack([_percore_from_out(res.results[k]["out"])
                            for k in range(NCORES)])

    return _head(percore, conv_b, fc1_w, fc1_b, fc2_w, fc2_b)


# --------------------------------------------------------------------------
# numpy self-check of the host-side math (no hardware needed)
# --------------------------------------------------------------------------

def _selfcheck():
    rng = np.random.default_rng(0)
    Tm = 12 * (NCOL + 2) + 64
    eeg = rng.standard_normal((16, T)).astype(np.float32)
    W1 = (rng.standard_normal((10, 16, 9)) * 0.1).astype(np.float32)

    # reference conv for a window
    k = 3
    s = _core_start(k)
    ref = np.zeros((10, TC))
    for j in range(9):
        ref += np.einsum('oc,ct->ot', W1[:, :, j],
                         eeg[:, s + j:s + j + TC])

    eegp = np.concatenate([eeg, np.zeros((16, 64), np.float32)], axis=1)
    plo_k, phi_k = _eeg_phases12(eegp, k)
    WA, WC, WB = _eeg_lhsT12(W1)

    # emulate the three passes
    got = np.zeros((120, NCOL))
    got += WA.T @ plo_k[:, 0:NCOL]
    got += WC.T @ plo_k[:, 1:NCOL + 1]
    got += WB.T @ phi_k[:, 0:NCOL]
    got_ot = got.reshape(10, 12, NCOL).transpose(0, 2, 1).reshape(10, TC)
    err = np.abs(got_ot - ref).max()
    print("eeg B=12 max err:", err)
    assert err < 2e-2, err

    # wav path check
    wav = rng.standard_normal(T).astype(np.float32)
    E = rng.standard_normal((10, 23)) * 0.1
    w_pad = np.concatenate([np.zeros(7, np.float32), wav,
                            np.zeros(64, np.float32)]).astype(np.float16)
    ph = _wav_phases(w_pad, k)
    L = _wav_lhsT(E)
    gotw = (L.T @ ph.astype(np.float64)).reshape(10, 12, NCOL)
    gotw = gotw.transpose(0, 2, 1).reshape(10, TC)
    refw = np.zeros((10, TC))
    wp = np.concatenate([np.zeros(7), wav.astype(np.float64)])
    for j in range(23):
        refw += np.outer(E[:, j], wp[s + j:s + j + TC])
    errw = np.abs(gotw - refw).max()
    print("wav max err:", errw)
    assert errw < 2e-2, errw
    print("selfcheck OK")


if __name__ == "__main__":
    _selfcheck()


# revision 10
# speedup vs baseline: 1.1745x; 1.1745x over previous
"""Trainium2 Bass kernel for nn_CNN_88098369175781.

Model: x[1,1,18,T=262144] -> wavA=x[...,0,:], eeg=x[...,1:17,:], wavB=x[...,17,:]
  wav streams: proj(1->16, pointwise) -> diagonal sinc filter bank (15 taps,
  pad 7) -> conv(16->10, 9 taps) + bias -> relu -> global max-pool.
  eeg stream:  conv(16->10, 9 taps) + bias -> relu -> global max-pool.
  concat -> sigmoid FC(30->30) -> sigmoid FC(30->2).

Device decomposition:
  * Wav streams fuse to ONE 1->10ch 23-tap conv (precomposed host-side).
  * Bias/relu commute past the global max; device computes convs + maxima.
  * eeg conv via B=12 polyphase: out[o, s+12m+dt], M=120=(o,dt), three
    accumulating passes: A (s'=r in [0,8), K=128), C (s'=12+r from the
    m+1 column, K=128), B (s'=8+r' in [8,12), K=64).
  * wav via 12-phase polyphase (K=36, M=120); the A and B streams run as
    row-tiled concurrent matmuls (tile_position (0,0) / (64,0)).
  * PSUM evacuation: ACT casts some banks to fp16 SBUF; GpSimd folds the
    stream tails into those casts in-place (tensor_tensor max); DVE
    tensor_tensor_reduce consumes a fresh PSUM bank and a cast tile per
    pass, writing per-row maxima into the output tile.
  * 8 cores split the time axis (overlapping chunks; overlap free for max).
  * Host combines per-core maxima and runs the tiny FC head.
"""

import os
import numpy as np

T = 262144
NOUT = T - 8            # 262136 valid conv output positions
NCORES = 8
KLEN = 15
SIGMA = 0.005

B12 = 12
NCOL = 2731             # phase columns per core (12*2731 = 32772 outputs)
TC = 12 * NCOL          # outputs per core

_NC_CACHE = {}
LAST_RESULT = None      # BassKernelResults of the most recent device run


# --------------------------------------------------------------------------
# host-side weight precompute
# --------------------------------------------------------------------------

def _sinc_rows(mu):
    """Diagonal rows of the reference's sinc_kernel: [16, 15] float64."""
    k = np.linspace(-1.0, 1.0, KLEN)
    kk = (k[None, :] - np.asarray(mu, np.float64)[:, None]) / SIGMA
    nos = np.sum(np.abs(kk) < 1e-5, axis=1)
    kk = np.where((nos >= 0.5)[:, None], kk - 5e-5, kk)
    return np.sin(np.pi * kk) / (np.pi * kk)


def _composite_wav_weights(mu, proj_w, conv_w_i):
    """Fused 1->10ch 23-tap kernel E[o, s] (float64)."""
    krn = _sinc_rows(mu)                                  # [16,15]
    a = np.asarray(proj_w, np.float64)[:, 0, 0]           # [16]
    W = np.asarray(conv_w_i, np.float64)                  # [10,16,9]
    E = np.zeros((10, 23))
    for j in range(9):
        E[:, j:j + 15] += np.einsum('oc,cm->om', W[:, :, j] * a[None, :], krn)
    return E


def _eeg_lhsT12(W1):
    """B=12 eeg weights: (W_A [128,120], W_C [128,120], W_B [64,120]).

    cols (o*12+dt); A rows (c*8+r): W1[o,c,r-dt]; C rows (c*8+r):
    W1[o,c,12+r-dt]; B rows (c*4+r'): W1[o,c,8+r'-dt]."""
    W1 = np.asarray(W1, np.float64)         # [10,16,9]
    WA = np.zeros((128, 120))
    WC = np.zeros((128, 120))
    WB = np.zeros((64, 120))
    for o in range(10):
        for dt in range(12):
            col = o * 12 + dt
            for c in range(16):
                for r in range(8):
                    j = r - dt
                    if 0 <= j < 9:
                        WA[c * 8 + r, col] = W1[o, c, j]
                    j = 12 + r - dt
                    if 0 <= j < 9:
                        WC[c * 8 + r, col] = W1[o, c, j]
                for rp in range(4):
                    j = 8 + rp - dt
                    if 0 <= j < 9:
                        WB[c * 4 + rp, col] = W1[o, c, j]
    return (WA.astype(np.float32), WC.astype(np.float32),
            WB.astype(np.float32))


def _wav_lhsT(E):
    """[36, 120]: row v*3+q, col o*12+dt, val E[o, 12q+v-dt]."""
    out = np.zeros((36, 120))
    v, q, o, dt = np.meshgrid(np.arange(12), np.arange(3), np.arange(10),
                              np.arange(12), indexing='ij')
    s = 12 * q + v - dt
    valid = (s >= 0) & (s < 23)
    out[(v * 3 + q)[valid], (o * 12 + dt)[valid]] = E[o[valid], np.clip(s[valid], 0, 22)]
    return out.astype(np.float32)


# --------------------------------------------------------------------------
# host-side per-core input slicing
# --------------------------------------------------------------------------

def _core_start(k):
    return min(k * 32767, NOUT - TC)


def _eeg_phases12(eegp, k):
    """eegp: [16, T+pad] fp16. Returns (plo [128, 2732], phi [64, 2731])."""
    s = _core_start(k)
    v = eegp[:, s:s + 12 * (NCOL + 1)]                    # [16, 32784]
    p = v.reshape(16, NCOL + 1, 12)                       # [16, 2732, 12]
    plo = p[:, :, 0:8].transpose(0, 2, 1).reshape(128, NCOL + 1)
    phi = p[:, 0:NCOL, 8:12].transpose(0, 2, 1).reshape(64, NCOL)
    return np.ascontiguousarray(plo), np.ascontiguousarray(phi)


def _wav_phases(w_pad, k):
    """[36, 2731]: row v*3+q, col n = w_pad[s + 12(n+q) + v]."""
    s = _core_start(k)
    sl = w_pad[s:s + 12 * (NCOL + 2)]
    y = sl.reshape(NCOL + 2, 12).T                        # y[v,m] = sl[12m+v]
    out = np.empty((36, NCOL), dtype=w_pad.dtype)
    for q in range(3):
        out[q::3, :] = y[:, q:q + NCOL]
    return np.ascontiguousarray(out)


# --------------------------------------------------------------------------
# bass kernel
# --------------------------------------------------------------------------

def _build_nc():
    import concourse.bacc as bacc
    import concourse.tile as tile
    import concourse.mybir as mybir

    f32 = mybir.dt.float32
    f16 = mybir.dt.float16
    Max = mybir.AluOpType.max
    X = mybir.AxisListType.X
    Copy = mybir.ActivationFunctionType.Copy

    no_ttr = bool(os.environ.get("KV_NO_TTR"))
    no_tilepos = bool(os.environ.get("KV_NO_TILEPOS"))
    nc = bacc.Bacc("TRN2", target_bir_lowering=False, debug=False,
                   num_devices=NCORES)

    # DRAM I/O
    wts = nc.dram_tensor("wts", [128, 480], f16, kind="ExternalInput")
    plo = nc.dram_tensor("plo", [128, NCOL + 1], f16, kind="ExternalInput")
    phi = nc.dram_tensor("phi", [64, NCOL], f16, kind="ExternalInput")
    wavA = nc.dram_tensor("wavA", [36, NCOL], f16, kind="ExternalInput")
    wavB = nc.dram_tensor("wavB", [36, NCOL], f16, kind="ExternalInput")
    out = nc.dram_tensor("out", [128, 8], f16, kind="ExternalOutput")

    NINIT = -60000.0
    N_WARM = 4

    with tile.TileContext(nc) as tc:
        with (
            tc.tile_pool(name="sb", bufs=1) as sb,
            tc.tile_pool(name="ps", bufs=4, space="PSUM") as psp,
        ):
            # ---- SBUF tiles
            scr = sb.tile([128, 512], f16, tag="scr")
            wtsT = sb.tile([128, 480], f16, tag="wtsT")
            ploT0 = sb.tile([128, 1025], f16, tag="ploT0")
            ploT1 = sb.tile([128, 1025], f16, tag="ploT1")
            ploT2 = sb.tile([128, 684], f16, tag="ploT2")
            phiT0 = sb.tile([64, 1024], f16, tag="phiT0")
            phiT1 = sb.tile([64, 1024], f16, tag="phiT1")
            phiT2 = sb.tile([64, 683], f16, tag="phiT2")
            wavTA = sb.tile([36, NCOL], f16, tag="wavTA")
            wavTB = sb.tile([128, NCOL], f16, tag="wavTB")  # rows 64..99 used
            cE0 = sb.tile([120, 1024], f16, tag="cE0")
            cE1 = sb.tile([120, 1024], f16, tag="cE1")
            cA0 = sb.tile([120, 1024], f16, tag="cA0")
            cA1 = sb.tile([120, 1024], f16, tag="cA1")
            fscr = sb.tile([120, 1024], f16, tag="fscr")
            fscr2 = sb.tile([120, 1024], f16, tag="fscr2")
            out32 = sb.tile([128, 8], f16, tag="out32")

            # ---- warmup scratch init first so PE can start immediately
            nc.gpsimd.memset(scr[:], 0.0)
            nc.gpsimd.memset(out32[:], NINIT)

            # ---- input DMAs.
            # SP ring: wts then eeg P_lo pieces (in matmul order).
            nc.sync.dma_start(wtsT[:], wts[:])
            nc.sync.dma_start(ploT0[:], plo[:, 0:1025])
            nc.sync.dma_start(ploT1[:], plo[:, 1024:2049])
            nc.sync.dma_start(ploT2[:], plo[:, 2048:2732])
            # ACT ring: phi pieces + wav streams (phi2 only needed late)
            nc.scalar.dma_start(phiT0[:], phi[:, 0:1024])
            nc.scalar.dma_start(phiT1[:], phi[:, 1024:2048])
            nc.scalar.dma_start(wavTA[:], wavA[:])
            nc.scalar.dma_start(wavTB[64:100, :], wavB[:])
            nc.scalar.dma_start(phiT2[:], phi[:, 2048:2731])

            # ---- PE warmup on scratch (keeps HAM busy while DMAs land)
            psD = psp.tile([120, 1024], f32, tag="u", name="psD")
            for _ in range(N_WARM):
                nc.tensor.matmul(psD[0:80, 0:512], scr[:, 0:80], scr[:],
                                 start=True, stop=True)

            wA = wtsT[:, 0:120]          # eeg pass A (K=128)
            wC = wtsT[:, 120:240]        # eeg pass C (K=128)
            wB = wtsT[0:64, 240:360]     # eeg pass B (K=64)
            wWA = wtsT[0:36, 360:480]    # wav A lhsT
            wWB = wtsT[64:100, 360:480]  # wav B lhsT

            def eeg_ac(ps, pl, lo, n):
                # pass-major: both A chunks, then both C chunks
                for j in range(0, n, 512):
                    w = min(512, n - j)
                    nc.tensor.matmul(ps[:, j:j + w], wA,
                                     pl[:, lo + j:lo + j + w],
                                     start=True, stop=False)
                for j in range(0, n, 512):
                    w = min(512, n - j)
                    nc.tensor.matmul(ps[:, j:j + w], wC,
                                     pl[:, lo + j + 1:lo + j + 1 + w],
                                     start=False, stop=False)

            def eeg_b(ps, ph, phlo, n):
                for j in range(0, n, 512):
                    w = min(512, n - j)
                    nc.tensor.matmul(ps[:, j:j + w], wB,
                                     ph[:, phlo + j:phlo + j + w],
                                     start=False, stop=True)

            def wav_pair(psA, psB, c0, n):
                tpA = None if no_tilepos else (0, 0)
                tpB = None if no_tilepos else (64, 0)
                for j in range(0, n, 512):
                    w = min(512, n - j)
                    nc.tensor.matmul(psA[:, j:j + w], wWA,
                                     wavTA[:, c0 + j:c0 + j + w],
                                     start=True, stop=True,
                                     tile_position=tpA)
                    nc.tensor.matmul(psB[:, j:j + w], wWB,
                                     wavTB[64:100, c0 + j:c0 + j + w],
                                     start=True, stop=True,
                                     tile_position=tpB)

            # ---- matmul schedule (PSUM pool rotates 4x [120,1024] tiles)
            psE0 = psp.tile([120, 1024], f32, tag="u", name="psE0")
            eeg_ac(psE0, ploT0, 0, 1024)
            eeg_b(psE0, phiT0, 0, 1024)
            nc.scalar.activation(cE0[:], psE0[:], Copy)

            psE1 = psp.tile([120, 1024], f32, tag="u", name="psE1")
            eeg_ac(psE1, ploT1, 0, 1024)
            eeg_b(psE1, phiT1, 0, 1024)
            nc.scalar.activation(cE1[:], psE1[:], Copy)

            psA0 = psp.tile([120, 1024], f32, tag="u", name="psA0")
            psB0 = psp.tile([120, 1024], f32, tag="u", name="psB0")
            wav_pair(psA0, psB0, 0, 1024)
            nc.scalar.activation(cA0[:], psA0[:], Copy)
            nc.vector.tensor_reduce(out32[0:120, 3:4], psB0[:], X, Max)

            psE2 = psp.tile([120, 1024], f32, tag="u", name="psE2")
            eeg_ac(psE2, ploT2, 0, 683)

            # eeg fold + reduce while E2/A1B1 matmuls run
            nc.vector.tensor_tensor(fscr[:], cE0[:], cE1[:], Max)
            nc.vector.tensor_reduce(out32[0:120, 0:1], fscr[:], X, Max)

            psA1 = psp.tile([120, 1024], f32, tag="u", name="psA1")
            psB1 = psp.tile([120, 1024], f32, tag="u", name="psB1")
            wav_pair(psA1, psB1, 1024, 1024)
            nc.scalar.activation(cA1[:], psA1[:], Copy)
            nc.vector.tensor_reduce(out32[0:120, 5:6], psB1[:], X, Max)

            eeg_b(psE2, phiT2, 0, 683)
            nc.vector.tensor_reduce(out32[0:120, 1:2], psE2[:, 0:683], X, Max)

            psA2 = psp.tile([120, 1024], f32, tag="u", name="psA2")
            psB2 = psp.tile([120, 1024], f32, tag="u", name="psB2")
            wav_pair(psA2, psB2, 2048, 683)

            # wavA fold + reduce; tails direct
            nc.vector.tensor_tensor(fscr2[:], cA0[:], cA1[:], Max)
            nc.vector.tensor_reduce(out32[0:120, 2:3], fscr2[:], X, Max)
            nc.vector.tensor_reduce(out32[0:120, 4:5], psA2[:, 0:683], X, Max)
            nc.vector.tensor_reduce(out32[0:120, 6:7], psB2[:, 0:683], X, Max)

            nc.sync.dma_start(out[:], out32[:])

    nc.compile()
    return nc


def _get_nc():
    if "nc" not in _NC_CACHE:
        _NC_CACHE["nc"] = _build_nc()
    return _NC_CACHE["nc"]


# --------------------------------------------------------------------------
# entry point
# --------------------------------------------------------------------------

def _prepare_in_maps(x, mu, projA_w, projB_w, conv_w):
    x = np.asarray(x, np.float32)
    eegp = np.concatenate([x[0, 0, 1:17, :], np.zeros((16, 64), np.float32)],
                          axis=1).astype(np.float16)
    zt = np.zeros(64, np.float32)
    w_padA = np.concatenate([np.zeros(7, np.float32), x[0, 0, 0, :], zt]
                            ).astype(np.float16)
    w_padB = np.concatenate([np.zeros(7, np.float32), x[0, 0, 17, :], zt]
                            ).astype(np.float16)

    conv_w = np.asarray(conv_w)
    E_A = _composite_wav_weights(mu, projA_w, conv_w[0])
    E_B = _composite_wav_weights(mu, projB_w, conv_w[2])
    WA, WC, WB = _eeg_lhsT12(conv_w[1])
    wts_np = np.zeros((128, 480), np.float16)
    wts_np[:, 0:120] = WA
    wts_np[:, 120:240] = WC
    wts_np[0:64, 240:360] = WB
    wts_np[0:36, 360:480] = _wav_lhsT(E_A)
    wts_np[64:100, 360:480] = _wav_lhsT(E_B)

    in_maps = []
    for k in range(NCORES):
        plo_k, phi_k = _eeg_phases12(eegp, k)
        in_maps.append({
            "wts": wts_np,
            "plo": plo_k,
            "phi": phi_k,
            "wavA": _wav_phases(w_padA, k),
            "wavB": _wav_phases(w_padB, k),
        })
    return in_maps


def _head(percore, conv_b, fc1_w, fc1_b, fc2_w, fc2_b):
    m = percore.max(axis=0).astype(np.float64)            # [360]
    eeg_o = m[0:120].reshape(10, 12).max(axis=1)
    wavA_o = m[120:240].reshape(10, 12).max(axis=1)
    wavB_o = m[240:360].reshape(10, 12).max(axis=1)
    conv_b = np.asarray(conv_b, np.float64)
    f = np.concatenate([np.maximum(wavA_o + conv_b[0], 0.0),
                        np.maximum(eeg_o + conv_b[1], 0.0),
                        np.maximum(wavB_o + conv_b[2], 0.0)])
    h = 1.0 / (1.0 + np.exp(-(f @ np.asarray(fc1_w, np.float64).T
                              + np.asarray(fc1_b, np.float64))))
    o = 1.0 / (1.0 + np.exp(-(h @ np.asarray(fc2_w, np.float64).T
                              + np.asarray(fc2_b, np.float64))))
    return o[None, :].astype(np.float32)


def _percore_from_out(arr):
    """Device 'out' [128,8] fp16 -> flat [360] (eeg 120, wavA 120, wavB 120).

    cols 0,1: eeg partials; 2,4: wavA; 3,5,6: wavB."""
    arr = np.asarray(arr, np.float32)
    return np.concatenate([arr[0:120, [0, 1]].max(axis=1),
                           arr[0:120, [2, 4]].max(axis=1),
                           arr[0:120, [3, 5, 6]].max(axis=1)])


def kernel(x, mu, projA_w, projB_w, conv_w, conv_b, fc1_w, fc1_b, fc2_w, fc2_b):
    global LAST_RESULT
    in_maps = _prepare_in_maps(x, mu, projA_w, projB_w, conv_w)
    nc = _get_nc()

    if os.environ.get("KERNEL_USE_SIM"):
        from concourse.bass_interp import CoreSim
        percore = np.zeros((NCORES, 360), np.float32)
        for k in range(NCORES):
            sim = CoreSim(nc)
            for name, arr in in_maps[k].items():
                sim.tensor(name)[:] = arr
            sim.simulate()
            percore[k] = _percore_from_out(sim.tensor("out"))
    else:
        from concourse.bass_utils import run_bass_kernel_spmd
        trace = bool(os.environ.get("KERNEL_TRACE"))
        res = run_bass_kernel_spmd(nc, in_maps, list(range(NCORES)),
                                   trace=trace)
        LAST_RESULT = res
        percore = np.stack([_percore_from_out(res.results[k]["out"])
                            for k in range(NCORES)])

    return _head(percore, conv_b, fc1_w, fc1_b, fc2_w, fc2_b)


# --------------------------------------------------------------------------
# numpy self-check of the host-side math (no hardware needed)
# --------------------------------------------------------------------------

def _selfcheck():
    rng = np.random.default_rng(0)
    Tm = 12 * (NCOL + 2) + 64
    eeg = rng.standard_normal((16, T)).astype(np.float32)
    W1 = (rng.standard_normal((10, 16, 9)) * 0.1).astype(np.float32)

    # reference conv for a window
    k = 3
    s = _core_start(k)
    ref = np.zeros((10, TC))
    for j in range(9):
        ref += np.einsum('oc,ct->ot', W1[:, :, j],
                         eeg[:, s + j:s + j + TC])

    eegp = np.concatenate([eeg, np.zeros((16, 64), np.float32)], axis=1)
    plo_k, phi_k = _eeg_phases12(eegp, k)
    WA, WC, WB = _eeg_lhsT12(W1)

    # emulate the three passes
    got = np.zeros((120, NCOL))
    got += WA.T @ plo_k[:, 0:NCOL]
    got += WC.T @ plo_k[:, 1:NCOL + 1]
    got += WB.T @ phi_k[:, 0:NCOL]
    got_ot = got.reshape(10, 12, NCOL).transpose(0, 2, 1).reshape(10, TC)
    err = np.abs(got_ot - ref).max()
    print("eeg B=12 max err:", err)
    assert err < 2e-2, err

    # wav path check
    wav = rng.standard_normal(T).astype(np.float32)
    E = rng.standard_normal((10, 23)) * 0.1
    w_pad = np.concatenate([np.zeros(7, np.float32), wav,
                            np.zeros(64, np.float32)]).astype(np.float16)
    ph = _wav_phases(w_pad, k)
    L = _wav_lhsT(E)
    gotw = (L.T @ ph.astype(np.float64)).reshape(10, 12, NCOL)
    gotw = gotw.transpose(0, 2, 1).reshape(10, TC)
    refw = np.zeros((10, TC))
    wp = np.concatenate([np.zeros(7), wav.astype(np.float64)])
    for j in range(23):
        refw += np.outer(E[:, j], wp[s + j:s + j + TC])
    errw = np.abs(gotw - refw).max()
    print("wav max err:", errw)
    assert errw < 2e-2, errw
    print("selfcheck OK")


if __name__ == "__main__":
    _selfcheck()
